# revision 26
# baseline (speedup 1.0000x reference)
"""Trainium2 Bass kernel for the CurrentLIF recurrent spiking network.

Column-sharded recurrent weights across 8 NeuronCores (512 postsynaptic
neurons per core); per-step spike exchange via an 8-core AllGather.

v2 optimizations over the f32 baseline:
- Weights stored fp16 with a 2^24 exponent pre-scale (fp16 matmul runs at
  1 moving-column/cycle vs fp32's 4, and its 11-bit mantissa keeps spike
  flips inside the 2e-2 gate; the 2^-24 unscale is folded into existing
  DVE ops on the PSUM results).
- The feed-forward drive is computed on-device: input-spike chunks are
  staged as matmul stationaries and accumulated into the SAME PSUM bank
  as the recurrent drive (tau_ff == tau_syn[0], so Iff rides inside J0).
  The FF matmuls depend only on prefetched input spikes, so they execute
  during the previous step's AllGather window.
- Dummy matmuls keep the PE HAM clock-gate warm across the AllGather
  window (otherwise every step's matmuls run at 1.2 GHz instead of 2.4).
- The LIF state update is fused (scalar_tensor_tensor) and split so only
  drive-sum -> v -> staged-spikes sits on the critical path; refractory
  bookkeeping, v-reset, J update and the output DMA overlap the
  collective.
"""
import os
import numpy as np

import concourse.bass as bass
import concourse.bacc as bacc
import concourse.tile as tile
import concourse.mybir as mybir
import concourse.bass_utils as bass_utils

F32 = mybir.dt.float32
F32R = mybir.dt.float32r
F16 = mybir.dt.float16
BF16 = mybir.dt.bfloat16
AL = mybir.AluOpType

DT = 1.0
BETA = float(np.float32(np.exp(-DT / 20.0)))
ALPHA0 = float(np.float32(np.exp(-DT / 5.0)))
ALPHA1 = float(np.float32(np.exp(-DT / 10.0)))
B = 16
NIN = 1024
N = 4096
NSH = 512
KC = 32
KF = 8

MODE = "f32r"        # "fp16" | "f32r" | "hilo"
ABL = os.environ.get("KERNEL_ABL", "")   # dev-only ablations: "noex","nomm",
SCALE = float(2.0 ** 24) if MODE == "fp16" else 1.0
UNSCALE = 1.0 / SCALE
NDUM = 0
PF = 3

_CACHE = {}


def _dtypes():
    if MODE == "fp16":
        return dict(w=F16, g=F16, sp=F16, stg=F16, cast_gload=False)
    if MODE == "f32r":
        return dict(w=F32R, g=F32R, sp=F32R, stg=BF16, cast_gload=True)
    if MODE == "hilo":
        return dict(w=BF16, g=BF16, sp=BF16, stg=BF16, cast_gload=False)
    raise ValueError(MODE)


def _build(T):
    dt = _dtypes()
    WDT, GDT, SPDT, STGDT = dt["w"], dt["g"], dt["sp"], dt["stg"]
    npass = 2 if MODE == "hilo" else 1

    nc = bacc.Bacc("TRN2", target_bir_lowering=False, debug=False, num_devices=8)
    Wt_d = [nc.dram_tensor(f"wt{i}", [128, KC * NSH], WDT, kind="ExternalInput")
            for i in range(npass)]
    WF_d = [nc.dram_tensor(f"wf{i}", [128, KF * NSH], WDT, kind="ExternalInput")
            for i in range(npass)]
    SP_d = nc.dram_tensor("sp", [T, 128, KF * 32], SPDT, kind="ExternalInput")
    MF_d = nc.dram_tensor("mf", [128, 128], F32, kind="ExternalInput")
    AD_d = nc.dram_tensor("adec", [128, 128], F32, kind="ExternalInput")
    ID_d = nc.dram_tensor("ident", [32, 32], F32, kind="ExternalInput")
    GZ_d = nc.dram_tensor("gz", [128, KC * 32], GDT, kind="ExternalInput")
    out_d = nc.dram_tensor("out", [T, 128, 64], F32, kind="ExternalOutput")

    with tile.TileContext(nc) as tc:
        with tc.tile_pool(name="big", bufs=1) as big, \
             tc.tile_pool(name="state", bufs=1) as state, \
             tc.tile_pool(name="work", bufs=3) as work, \
             tc.tile_pool(name="ffp", bufs=PF + 1) as ffp, \
             tc.tile_pool(name="psA", bufs=1, space="PSUM") as psA, \
             tc.tile_pool(name="psB", bufs=1, space="PSUM") as psB, \
             tc.tile_pool(name="psC", bufs=1, space="PSUM") as psC, \
             tc.tile_pool(name="dram", bufs=1, space="DRAM") as dram:

            Wt = [big.tile([128, KC * NSH], WDT, name=f"Wt{i}") for i in range(npass)]
            WF = [big.tile([128, KF * NSH], WDT, name=f"WF{i}") for i in range(npass)]
            for i in range(npass):
                nc.sync.dma_start(Wt[i][:], Wt_d[i][:])
                nc.sync.dma_start(WF[i][:], WF_d[i][:])

            G = [big.tile([128, KC * 32], GDT, name=f"G{i}", tag=f"G{i}")
                 for i in range(2)]
            STG = [state.tile([128, 128], STGDT, name=f"stg{i}", tag=f"stg{i}")
                   for i in range(2)]
            if ABL == "mindve":
                for st_ in STG:
                    nc.gpsimd.memset(st_[:], 0.0)

            MF = state.tile([128, 128], F32, name="MF")
            Adec = state.tile([128, 128], F32, name="Adec")
            ident = state.tile([32, 32], F32, name="ident")
            nc.sync.dma_start(MF[:], MF_d[:])
            nc.sync.dma_start(Adec[:], AD_d[:])
            nc.sync.dma_start(ident[:], ID_d[:])

            J = state.tile([128, 128], F32, name="J")
            Jsum = state.tile([128, 64], F32, name="Jsum")
            v = state.tile([128, 64], F32, name="v")
            refr = state.tile([128, 64], F32, name="refr")
            am = state.tile([128, 64], F32, name="am")
            vx = state.tile([128, 64], F32, name="vx")
            sring = state.tile([128, 256], F32, name="sring")
            ns = state.tile([128, 64], F32, name="ns")
            for t_ in (J, Jsum, v, refr, sring, ns, vx):
                nc.gpsimd.memset(t_[:], 0.0)
            nc.gpsimd.memset(am[:], 1.0)
            for g_ in G:
                nc.sync.dma_start(g_[:], GZ_d[:])

            ag_in = [dram.tile([128, 4, 32], STGDT, name=f"agin{i}", tag=f"agin{i}")
                     for i in range(2)]
            ag_out = [dram.tile([8, 128, 4, 32], STGDT, name=f"agout{i}",
                                tag=f"agout{i}")
                      for i in range(2)]

            NB = (T + 3) // 4
            sp_blocks = {}

            def _fetch_block(b4):
                n = min(4, T - 4 * b4)
                ft = ffp.tile([128, 4 * KF * 32], SPDT, name=f"spb{b4}",
                              tag="sp")
                nc.sync.dma_start(
                    ft[:, 0:n * KF * 32].rearrange("p (k c) -> p k c", k=n),
                    SP_d[:][4 * b4:4 * b4 + n].rearrange("k p c -> p k c"))
                sp_blocks[b4] = ft

            for b4 in range(min(3, NB)):
                _fetch_block(b4)

            for t in range(T):
                par = t % 2
                g = G[par]
                stg = STG[par]
                blk = sp_blocks[t // 4]
                spt = blk[:, (t % 4) * KF * 32:(t % 4 + 1) * KF * 32]

                # --- hidden under previous exchange: J decay + row-sum ---
                nc.vector.tensor_tensor(J[:], J[:], Adec[:], AL.mult)
                Jr = J[:].rearrange("p (q s b) -> p q s b", q=4, s=2)
                nc.vector.tensor_tensor(Jsum[:].rearrange("p (q b) -> p q b", q=4),
                                        Jr[:, :, 0, :], Jr[:, :, 1, :], AL.add)
                # vx = beta*v + Jsum precomputed off the critical path
                nc.vector.scalar_tensor_tensor(vx[:], v[:], BETA, Jsum[:],
                                               AL.mult, AL.add)

                # --- FF drive: runs during the previous exchange (only
                # depends on prefetched input spikes + free PSUM bank) ---
                dr = psA.tile([32, NSH], F32, name=f"dr{par}", tag=f"dr{par}")
                for i in range(npass):
                    for c in range(KF):
                        nc.tensor.matmul(dr[:], spt[:, 32 * c:32 * c + 32],
                                         WF[i][:, NSH * c:NSH * c + NSH],
                                         start=(i == 0 and c == 0), stop=False)

                # --- recurrent drive (waits on gathered spikes) ---
                nrec = 1 if ABL == "nomm" else KC
                for i in range(npass):
                    for k in range(nrec):
                        nc.tensor.matmul(dr[:], g[:, 32 * k:32 * k + 32],
                                         Wt[i][:, NSH * k:NSH * k + NSH],
                                         start=False,
                                         stop=(i == npass - 1 and k == nrec - 1))

                if ABL == "mindve":
                    # only the MM chain + exchange remain; PSUM result unread
                    drc = work.tile([32, NSH], F32, name="drc", tag="drc")
                    nc.scalar.copy(drc[:], dr[:])
                    if t < T - 1:
                        nc.sync.dma_start(
                            ag_in[par][:],
                            stg[:].rearrange("p (q c) -> p q c", q=4))
                        nc.gpsimd.collective_compute(
                            "AllGather", AL.bypass,
                            replica_groups=[list(range(8))],
                            ins=[ag_in[par].opt()], outs=[ag_out[par].opt()])
                        gn = G[1 - par]
                        nc.gpsimd.dma_start(
                            gn[:].rearrange("p (r x) -> p r x", r=8),
                            ag_out[par][:].rearrange("r p q c -> p r (q c)"))
                    nc.scalar.dma_start(out_d[:][t], sring[:, 0:64])
                    if t + PF < T:
                        ft = ffp.tile([128, KF * 32], SPDT, name=f"sp{t+PF}",
                                      tag="sp")
                        nc.sync.dma_start(ft[:], SP_d[:][t + PF])
                        sp_tiles[t + PF] = ft
                    continue
                drc = work.tile([32, NSH], F32, name="drc", tag="drc")
                nc.scalar.copy(drc[:], dr[:])
                tp = psB.tile([128, 128], F32, name=f"tp{par}", tag=f"tp{par}")
                for q in range(4):
                    nc.tensor.transpose(tp[:, 32 * q:32 * q + 32],
                                        drc[:, 128 * q:128 * q + 128], ident[:])

                # --- critical path: drive sum -> v -> staged spikes ---
                # (each op reads PSUM at most once: single DVE PSUM read port)
                tpr = tp[:].rearrange("p (q s b) -> p q s b", q=4, s=2)
                dsum = work.tile([128, 64], F32, name="dsum", tag="dsum")
                nc.vector.scalar_tensor_tensor(
                    dsum[:].rearrange("p (q b) -> p q b", q=4),
                    tpr[:, :, 0, :], UNSCALE,
                    vx[:].rearrange("p (q b) -> p q b", q=4),
                    AL.mult, AL.add)
                nc.vector.scalar_tensor_tensor(
                    v[:].rearrange("p (q b) -> p q b", q=4),
                    tpr[:, :, 1, :], UNSCALE,
                    dsum[:].rearrange("p (q b) -> p q b", q=4),
                    AL.mult, AL.add)
                nc.vector.tensor_tensor(v[:], v[:], am[:], AL.mult)

                sr = stg[:].rearrange("p (q s b) -> p q s b", q=4, s=2)
                mfr = MF[:].rearrange("p (q s b) -> p q s b", q=4, s=2)
                vr = v[:].rearrange("p (q b) -> p q b", q=4)
                nc.vector.scalar_tensor_tensor(
                    sr[:, :, 0, :], vr, 1.0, mfr[:, :, 0, :], AL.is_gt, AL.mult)
                nc.vector.scalar_tensor_tensor(
                    sr[:, :, 1, :], vr, 1.0, mfr[:, :, 1, :], AL.is_gt, AL.mult)

                if t < T - 1 and ABL != "noex":
                    nc.sync.dma_start(ag_in[par][:],
                                      stg[:].rearrange("p (q c) -> p q c", q=4))
                    if ABL != "nocoll":
                        nc.gpsimd.collective_compute(
                            "AllGather", AL.bypass,
                            replica_groups=[list(range(8))],
                            ins=[ag_in[par].opt()], outs=[ag_out[par].opt()])
                    gn = G[1 - par]
                    eng = nc.gpsimd if _dtypes()["cast_gload"] else nc.sync
                    eng.dma_start(
                        gn[:].rearrange("p (r x) -> p r x", r=8),
                        ag_out[par][:].rearrange("r p q c -> p r (q c)"))

                # --- overlapped with the exchange ---
                sl = t % 4
                s = sring[:, 64 * sl:64 * sl + 64]
                nc.vector.tensor_scalar(s, v[:], 1.0, None, AL.is_gt)
                if sl == 3 or t == T - 1:
                    t0 = t - sl
                    nc.scalar.dma_start(
                        out_d[:][t0:t + 1].rearrange("k p x -> p k x"),
                        sring[:, 0:64 * (sl + 1)].rearrange(
                            "p (k x) -> p k x", k=sl + 1))
                nc.vector.scalar_tensor_tensor(J[:], tp[:], UNSCALE, J[:],
                                               AL.mult, AL.add)
                nc.vector.tensor_scalar(ns[:], s, -1.0, 1.0, AL.mult, AL.add)
                nc.vector.tensor_tensor(v[:], v[:], ns[:], AL.mult)
                sp_ = sring[:, 64 * ((t - 1) % 4):64 * ((t - 1) % 4) + 64]
                nc.vector.tensor_tensor(refr[:], s, sp_, AL.add)
                nc.vector.tensor_scalar(am[:], refr[:], 0.0, None, AL.is_le)

                if t < T - 1:
                    scr = psC.tile([32, NSH], F32, name="scr", tag="scr")
                    for _ in range(NDUM):
                        nc.tensor.matmul(scr[:], g[:, 0:32],
                                         Wt[0][:, 0:NSH],
                                         start=True, stop=True)

                if t % 4 == 3:
                    done = t // 4
                    if done in sp_blocks:
                        del sp_blocks[done]
                    nxt = done + 3
                    if nxt < NB and nxt not in sp_blocks:
                        _fetch_block(nxt)
    nc.compile()
    return nc


def _to_np16(x):
    return (np.asarray(x, np.float32) * np.float32(SCALE)).astype(np.float16)


def _bf16_trunc(x):
    import ml_dtypes
    return np.asarray(x, np.float32).astype(ml_dtypes.bfloat16).astype(np.float32)


def _w_passes(Wp):
    """Returns list of np arrays (one per matmul pass) in storage dtype."""
    if MODE == "fp16":
        return [_to_np16(Wp)]
    if MODE == "f32r":
        return [np.asarray(Wp, np.float32)]
    if MODE == "hilo":
        import ml_dtypes
        hi = _bf16_trunc(Wp)
        lo = np.asarray(Wp, np.float32) - hi
        return [hi.astype(ml_dtypes.bfloat16), lo.astype(ml_dtypes.bfloat16)]
    raise ValueError(MODE)


def _sp_np(x):
    if MODE == "fp16":
        return x.astype(np.float16)
    if MODE == "f32r":
        return x.astype(np.float32)
    import ml_dtypes
    return x.astype(ml_dtypes.bfloat16)


def _prep_inputs(input_spikes, W, W_FF, cell_type_indices, T):
    beta = np.float32(BETA)
    Wp = ((np.float32(1.0) - beta) * np.asarray(W, np.float32))
    WFFp = ((np.float32(1.0) - beta) * np.asarray(W_FF, np.float32))
    B_, Tf, NIN_ = input_spikes.shape
    # input spikes staged as matmul stationaries: SP[t, p, (c, s, b)] with
    # the s=1 half zero so the first FF matmul covers all 32 PSUM rows
    spt = np.asarray(input_spikes, np.float32).transpose(1, 2, 0)[:T]  # [T,NIN,B]
    SPfull = np.zeros((T, 128, KF, 2, 16), np.float32)
    SPfull[:, :, :, 0, :] = spt.reshape(T, KF, 128, B_).transpose(0, 2, 1, 3)
    SPfull = _sp_np(SPfull.reshape(T, 128, KF * 32))
    cti = np.asarray(cell_type_indices).astype(np.int32)
    ident = np.eye(32, dtype=np.float32)
    alphas = np.zeros((128, 4, 2, 16), np.float32)
    alphas[:, :, 0, :] = ALPHA0
    alphas[:, :, 1, :] = ALPHA1
    alphas = alphas.reshape(128, 128)
    gz = np.zeros((128, KC * 32), np.float32)
    if MODE == "fp16":
        gz = gz.astype(np.float16)
    elif MODE == "hilo":
        import ml_dtypes
        gz = gz.astype(ml_dtypes.bfloat16)
    in_maps = []
    for c in range(8):
        Wc = Wp[:, 512 * c:512 * (c + 1)]
        Wt = Wc.reshape(32, 128, 512).transpose(1, 0, 2).reshape(128, 32 * 512)
        WFc = WFFp[:, 512 * c:512 * (c + 1)]
        WFt = WFc.reshape(KF, 128, 512).transpose(1, 0, 2).reshape(128, KF * 512)
        ctic = cti[512 * c:512 * (c + 1)].reshape(4, 128)
        m0 = (ctic == 0).astype(np.float32)
        m1 = -(ctic == 1).astype(np.float32)
        MFa = np.zeros((128, 4, 2, 16), np.float32)
        MFa[:, :, 0, :] = m0.T[:, :, None]
        MFa[:, :, 1, :] = m1.T[:, :, None]
        MF = MFa.reshape(128, 128).copy()
        m = {"sp": SPfull, "mf": MF, "adec": alphas, "ident": ident,
             "gz": gz}
        for i, wp in enumerate(_w_passes(Wt)):
            m[f"wt{i}"] = wp
        for i, wp in enumerate(_w_passes(WFt)):
            m[f"wf{i}"] = wp
        in_maps.append(m)
    return in_maps


def _assemble(results, T):
    cols = []
    for c in range(8):
        arr = results[c]["out"].reshape(T, 128, 4, 16)
        cols.append(arr.transpose(3, 0, 2, 1).reshape(B, T, 512))
    return np.concatenate(cols, axis=2).astype(np.float32)


def kernel(input_spikes, W, W_FF, cell_type_indices):
    T = int(input_spikes.shape[1])
    if T not in _CACHE:
        _CACHE[T] = _build(T)
    nc = _CACHE[T]
    in_maps = _prep_inputs(np.asarray(input_spikes), np.asarray(W),
                           np.asarray(W_FF), np.asarray(cell_type_indices), T)
    res = bass_utils.run_bass_kernel_spmd(nc, in_maps, core_ids=list(range(8)))
    return _assemble(res.results, T)
